# revision 18
# baseline (speedup 1.0000x reference)
"""Differentiable K-means (VQ codebook) forward on 8 TRN2 NeuronCores.

x: [16, 8192, 64] f32, centroids: [512, 64] f32
out[n] = softmax_k(-(|x_n - c_k|^2)/T) @ C, T = 0.1

Math: softmax_k(-(x^2 - 2 x.c + c^2)/T) == softmax_k((2 x.c - c^2)/T)
(the x^2 term cancels). Device computes E[k,n] = exp((2/T) * (x_n . c_k)) on
the ScalarE; the per-cluster factor W_k = exp(-c_k^2/T) is folded into the
second matmul's moving operand: out_aug[n,:] = sum_k E[k,n] * W_k * [C_k | 1],
giving the unnormalized mixture (cols 0..63) and the softmax denominator
(col 64) in one pass. out = cols0..63 / col64.

Device dataflow (per core, 16384 points):
- host pre-transposes the x shard to xT [64, 16384] (column-permuted so that
  within each 512-pt tile, col q*128+j = point 4j+q -> both input and output
  DMAs are 1KB-contiguous per partition). Host also prepares the centroid
  constants (vertically-duplicated cT, bf16 weighted-augmented centroids).
- xT tiles DMA'd twice (rows 0:64 / 64:128) so mm1 runs K=64 f32r matmuls
  row-packed two-at-a-time in PE row groups 0/64 (2x matmul throughput).
- mm1: cross^T chunks [128 clusters, 512 pts] in PSUM -> exp on ScalarE
  (bf16 out) -> mm2 with E slices as stationary (bf16 FWL weight path) and
  [w*C_c | w] as 65-wide moving operand -> natural [128 pts, 65] PSUM.
- reciprocal + multiply on VectorE, contiguous DMA out via SWDGE.

Sharding: data-parallel on the flattened point axis (131072 -> 8 x 16384),
centroids replicated. No cross-core comms.
"""

from contextlib import ExitStack

import ml_dtypes
import numpy as np

import concourse.bass as bass
import concourse.tile as tile
from concourse import bacc, mybir
from concourse._compat import with_exitstack
from concourse.bass_utils import run_bass_kernel_spmd

N_CORES = 8
N_PTS = 16384  # points per core
K = 512  # clusters
D = 64  # feature dim
TEMP = 0.1
TILE_PTS = 512  # points per inner tile
QS = TILE_PTS // 128  # 4 point-subgroups per tile
KC = K // 128  # 4 cluster chunks

F32 = mybir.dt.float32
F32R = mybir.dt.float32r
BF16 = mybir.dt.bfloat16


@with_exitstack
def _kmeans_body(ctx: ExitStack, tc: tile.TileContext, out_ap, xt_ap, ct2_ap, cw_ap, n_pts):
    nc = tc.nc
    n_tiles = n_pts // TILE_PTS

    # out[(t p q), d] -> [t, p, q*d]; partition p holds 4 consecutive points
    # (1KB contiguous per partition). xT columns are host-permuted to match:
    # xT col t*512 + q*128 + j  =  point t*512 + 4j + q.
    out_r = out_ap.rearrange("(t p q) d -> t p (q d)", p=128, q=QS)

    consts = ctx.enter_context(tc.tile_pool(name="consts", bufs=1))
    small = ctx.enter_context(tc.tile_pool(name="small", bufs=3))
    xtp = ctx.enter_context(tc.tile_pool(name="xtp", bufs=3))
    epool = ctx.enter_context(tc.tile_pool(name="epool", bufs=2))
    outp = ctx.enter_context(tc.tile_pool(name="outp", bufs=3))
    ps_cr = ctx.enter_context(tc.tile_pool(name="ps_cr", bufs=3, space="PSUM"))
    ps_fin = ctx.enter_context(tc.tile_pool(name="ps_fin", bufs=2, space="PSUM"))

    # constants, host-precomputed; loaded on the SWDGE ring so they don't
    # queue behind the first xT loads on the HWDGE sequencer
    ct2_sb = consts.tile([128, K], F32R)
    nc.gpsimd.dma_start(ct2_sb, ct2_ap)
    cw = consts.tile([128, KC * (D + 1)], BF16)
    nc.gpsimd.dma_start(cw, cw_ap)

    def load_xt(t, ntile):
        xt2 = xtp.tile([128, 2 * TILE_PTS], F32R, tag="xt2", name=f"xt2_{t}")
        span = xt_ap[:, t * TILE_PTS : (t + ntile) * TILE_PTS]
        nc.sync.dma_start(xt2[0:64, 0 : ntile * TILE_PTS], span)
        nc.sync.dma_start(xt2[64:128, 0 : ntile * TILE_PTS], span)
        return xt2

    def mm1_exp(t, xt2, toff):
        e_sb = epool.tile([128, KC * TILE_PTS], BF16, tag="e", name=f"e_{t}")
        for pair in range(KC // 2):
            cr_ps = ps_cr.tile([128, 2 * TILE_PTS], F32, tag="cr", name=f"cr_{t}_{pair}")
            for h in range(2):
                c = pair * 2 + h
                nc.tensor.matmul(
                    cr_ps[:, h * TILE_PTS : (h + 1) * TILE_PTS],
                    lhsT=ct2_sb[h * 64 : (h + 1) * 64, c * 128 : (c + 1) * 128],
                    rhs=xt2[h * 64 : (h + 1) * 64, toff : toff + TILE_PTS],
                    start=True,
                    stop=True,
                )
            nc.scalar.activation(
                e_sb[:, pair * 2 * TILE_PTS : (pair + 1) * 2 * TILE_PTS],
                cr_ps,
                mybir.ActivationFunctionType.Exp,
                scale=2.0 / TEMP,
            )
        return e_sb

    def mm2_norm(t, e_sb, o2_t, slot):
        fin_ps = ps_fin.tile([128, QS * (D + 1)], F32, tag="fin", name=f"fin_{t}")
        for q in range(QS):
            for c in range(KC):
                nc.tensor.matmul(
                    fin_ps[:, q * (D + 1) : (q + 1) * (D + 1)],
                    lhsT=e_sb[
                        :, c * TILE_PTS + q * 128 : c * TILE_PTS + (q + 1) * 128
                    ],
                    rhs=cw[:, c * (D + 1) : (c + 1) * (D + 1)],
                    start=(c == 0),
                    stop=(c == KC - 1),
                )
        fin3 = fin_ps[:].rearrange("p (q e) -> p q e", e=D + 1)
        inv = small.tile([128, QS], F32, tag="inv", name=f"inv_{t}")
        nc.vector.reciprocal(inv, fin3[:, :, D])
        o3 = o2_t[:, slot, :].rearrange("p (q d) -> p q d", d=D)
        nc.vector.tensor_mul(o3, fin3[:, :, 0:D], inv[:].broadcast_to([128, QS, D]))

    # xt load groups: tile 0 alone (so mm1(0) starts after 256KB, not 512KB),
    # then pairs; output store groups: pairs, with the last two tiles single
    # (so the final DMA is small and starts early).
    assert n_tiles >= 4 and n_tiles % 2 == 0
    load_at = {t: 2 for t in range(0, n_tiles, 2)}
    store_group = {}
    for b in range(0, n_tiles - 2, 2):
        store_group[b] = (b, 2)
        store_group[b + 1] = (b, 2)
    store_group[n_tiles - 2] = (n_tiles - 2, 1)
    store_group[n_tiles - 1] = (n_tiles - 1, 1)

    # main loop, software-pipelined one tile deep: mm1/exp of tile t+1 is
    # emitted before mm2/normalize of tile t so the PE keeps feeding ScalarE.
    xt_bufs = {}

    def get_xt(t):
        if t in load_at:
            xt_bufs.clear()
            for i in range(load_at[t]):
                xt_bufs[t + i] = (load_xt(t, load_at[t]), i * TILE_PTS)
        return xt_bufs[t]

    o_bufs = {}

    def put_out(t, e_sb):
        base, width = store_group[t]
        if t == base:
            o_bufs[base] = outp.tile(
                [128, width, QS * D], F32, tag="o2", name=f"o2_{t}"
            )
        o2_t = o_bufs[base]
        mm2_norm(t, e_sb, o2_t, t - base)
        if t == base + width - 1:
            nc.gpsimd.dma_start(
                out_r[base : base + width].rearrange("a p n -> p a n"), o2_t
            )

    xt2, toff = get_xt(0)
    e_prev = mm1_exp(0, xt2, toff)
    for t in range(1, n_tiles + 1):
        if t < n_tiles:
            xt2, toff = get_xt(t)
            e_cur = mm1_exp(t, xt2, toff)
        put_out(t - 1, e_prev)
        if t < n_tiles:
            e_prev = e_cur


def build_nc(n_pts=N_PTS, debug=False):
    nc = bacc.Bacc("TRN2", target_bir_lowering=False, debug=debug, num_devices=N_CORES)
    xt_in = nc.dram_tensor("xt", [D, n_pts], F32R, kind="ExternalInput").ap()
    ct2_in = nc.dram_tensor("ct2", [128, K], F32R, kind="ExternalInput").ap()
    cw_in = nc.dram_tensor("cw", [128, KC * (D + 1)], BF16, kind="ExternalInput").ap()
    out = nc.dram_tensor("out", [n_pts, D], F32, kind="ExternalOutput").ap()
    with tile.TileContext(nc) as tc:
        _kmeans_body(tc, out, xt_in, ct2_in, cw_in, n_pts)
    nc.compile()
    return nc


def _host_xt(x_shard: np.ndarray) -> np.ndarray:
    """[n, 64] -> column-permuted transpose [64, n]:
    xT[d, t*512 + q*128 + j] = x[t*512 + 4j + q, d]."""
    n = x_shard.shape[0]
    xs = x_shard.reshape(n // TILE_PTS, 128, QS, D)
    return np.ascontiguousarray(xs.transpose(3, 0, 2, 1).reshape(D, n))


def _host_consts(centroids: np.ndarray):
    c = centroids.astype(np.float64)
    ct2 = np.concatenate([centroids.T, centroids.T], axis=0).astype(np.float32)
    w = np.exp(-(c * c).sum(-1) / TEMP)  # [K]
    aug = np.concatenate([c * w[:, None], w[:, None]], axis=1)  # [K, 65]
    cw = (
        aug.reshape(KC, 128, D + 1)
        .transpose(1, 0, 2)
        .reshape(128, KC * (D + 1))
        .astype(ml_dtypes.bfloat16)
    )
    return np.ascontiguousarray(ct2), np.ascontiguousarray(cw)


_NC_CACHE = None


def kernel(x: np.ndarray, centroids: np.ndarray) -> np.ndarray:
    global _NC_CACHE
    orig_shape = x.shape
    xf = x.reshape(-1, D).astype(np.float32, copy=False)
    cf = centroids.astype(np.float32, copy=False)
    n_total = xf.shape[0]
    assert n_total == N_CORES * N_PTS, n_total

    if _NC_CACHE is None:
        _NC_CACHE = build_nc()
    nc = _NC_CACHE

    ct2, cw = _host_consts(cf)
    in_maps = [
        {"xt": _host_xt(xf[i * N_PTS : (i + 1) * N_PTS]), "ct2": ct2, "cw": cw}
        for i in range(N_CORES)
    ]
    res = run_bass_kernel_spmd(nc, in_maps, core_ids=list(range(N_CORES)))
    out = np.concatenate([res.results[i]["out"] for i in range(N_CORES)], axis=0)
    return out.reshape(orig_shape).astype(x.dtype, copy=False)


# revision 19
# speedup vs baseline: 1.0087x; 1.0087x over previous
"""Differentiable K-means (VQ codebook) forward on 8 TRN2 NeuronCores.

x: [16, 8192, 64] f32, centroids: [512, 64] f32
out[n] = softmax_k(-(|x_n - c_k|^2)/T) @ C, T = 0.1

Math: softmax_k(-(x^2 - 2 x.c + c^2)/T) == softmax_k((2 x.c - c^2)/T)
(the x^2 term cancels). Device computes E[k,n] = exp((2/T) * (x_n . c_k)) on
the ScalarE; the per-cluster factor W_k = exp(-c_k^2/T) is folded into the
second matmul's moving operand: out_aug[n,:] = sum_k E[k,n] * W_k * [C_k | 1],
giving the unnormalized mixture (cols 0..63) and the softmax denominator
(col 64) in one pass. out = cols0..63 / col64.

Device dataflow (per core, 16384 points):
- host pre-transposes the x shard to xT [64, 16384] (column-permuted so that
  within each 512-pt tile, col q*128+j = point 4j+q -> both input and output
  DMAs are 1KB-contiguous per partition). Host also prepares the centroid
  constants (vertically-duplicated cT, bf16 weighted-augmented centroids).
- xT tiles DMA'd twice (rows 0:64 / 64:128) so mm1 runs K=64 f32r matmuls
  row-packed two-at-a-time in PE row groups 0/64 (2x matmul throughput).
- mm1: cross^T chunks [128 clusters, 512 pts] in PSUM -> exp on ScalarE
  (bf16 out) -> mm2 with E slices as stationary (bf16 FWL weight path) and
  [w*C_c | w] as 65-wide moving operand -> natural [128 pts, 65] PSUM.
- reciprocal + multiply on VectorE, contiguous DMA out via SWDGE.

Sharding: data-parallel on the flattened point axis (131072 -> 8 x 16384),
centroids replicated. No cross-core comms.
"""

from contextlib import ExitStack

import ml_dtypes
import numpy as np

import concourse.bass as bass
import concourse.tile as tile
from concourse import bacc, mybir
from concourse._compat import with_exitstack
from concourse.bass_utils import run_bass_kernel_spmd

N_CORES = 8
N_PTS = 16384  # points per core
K = 512  # clusters
D = 64  # feature dim
TEMP = 0.1
TILE_PTS = 512  # points per inner tile
QS = TILE_PTS // 128  # 4 point-subgroups per tile
KC = K // 128  # 4 cluster chunks

F32 = mybir.dt.float32
F32R = mybir.dt.float32r
BF16 = mybir.dt.bfloat16


@with_exitstack
def _kmeans_body(ctx: ExitStack, tc: tile.TileContext, out_ap, xt_ap, ct2_ap, cw_ap, n_pts):
    nc = tc.nc
    n_tiles = n_pts // TILE_PTS

    # out[(t p q), d] -> [t, p, q*d]; partition p holds 4 consecutive points
    # (1KB contiguous per partition). xT columns are host-permuted to match:
    # xT col t*512 + q*128 + j  =  point t*512 + 4j + q.
    out_r = out_ap.rearrange("(t p q) d -> t p (q d)", p=128, q=QS)

    consts = ctx.enter_context(tc.tile_pool(name="consts", bufs=1))
    small = ctx.enter_context(tc.tile_pool(name="small", bufs=3))
    xtp = ctx.enter_context(tc.tile_pool(name="xtp", bufs=3))
    epool = ctx.enter_context(tc.tile_pool(name="epool", bufs=2))
    outp = ctx.enter_context(tc.tile_pool(name="outp", bufs=3))
    ps_cr = ctx.enter_context(tc.tile_pool(name="ps_cr", bufs=3, space="PSUM"))
    ps_fin = ctx.enter_context(tc.tile_pool(name="ps_fin", bufs=2, space="PSUM"))

    # constants, host-precomputed; loaded on the SWDGE ring so they don't
    # queue behind the first xT loads on the HWDGE sequencer
    ct2_sb = consts.tile([128, K], F32R)
    nc.sync.dma_start(ct2_sb, ct2_ap)
    cw = consts.tile([128, KC * (D + 1)], BF16)
    nc.sync.dma_start(cw, cw_ap)

    def load_xt(t, ntile):
        xt2 = xtp.tile([128, 2 * TILE_PTS], F32R, tag="xt2", name=f"xt2_{t}")
        span = xt_ap[:, t * TILE_PTS : (t + ntile) * TILE_PTS]
        nc.sync.dma_start(xt2[0:64, 0 : ntile * TILE_PTS], span)
        nc.sync.dma_start(xt2[64:128, 0 : ntile * TILE_PTS], span)
        return xt2

    def mm1_exp(t, xt2, toff):
        e_sb = epool.tile([128, KC * TILE_PTS], BF16, tag="e", name=f"e_{t}")
        for pair in range(KC // 2):
            cr_ps = ps_cr.tile([128, 2 * TILE_PTS], F32, tag="cr", name=f"cr_{t}_{pair}")
            for h in range(2):
                c = pair * 2 + h
                nc.tensor.matmul(
                    cr_ps[:, h * TILE_PTS : (h + 1) * TILE_PTS],
                    lhsT=ct2_sb[h * 64 : (h + 1) * 64, c * 128 : (c + 1) * 128],
                    rhs=xt2[h * 64 : (h + 1) * 64, toff : toff + TILE_PTS],
                    start=True,
                    stop=True,
                )
            nc.scalar.activation(
                e_sb[:, pair * 2 * TILE_PTS : (pair + 1) * 2 * TILE_PTS],
                cr_ps,
                mybir.ActivationFunctionType.Exp,
                scale=2.0 / TEMP,
            )
        return e_sb

    def mm2_norm(t, e_sb, o2_t, slot):
        fin_ps = ps_fin.tile([128, QS * (D + 1)], F32, tag="fin", name=f"fin_{t}")
        for q in range(QS):
            for c in range(KC):
                nc.tensor.matmul(
                    fin_ps[:, q * (D + 1) : (q + 1) * (D + 1)],
                    lhsT=e_sb[
                        :, c * TILE_PTS + q * 128 : c * TILE_PTS + (q + 1) * 128
                    ],
                    rhs=cw[:, c * (D + 1) : (c + 1) * (D + 1)],
                    start=(c == 0),
                    stop=(c == KC - 1),
                )
        fin3 = fin_ps[:].rearrange("p (q e) -> p q e", e=D + 1)
        inv = small.tile([128, QS], F32, tag="inv", name=f"inv_{t}")
        nc.vector.reciprocal(inv, fin3[:, :, D])
        o3 = o2_t[:, slot, :].rearrange("p (q d) -> p q d", d=D)
        nc.vector.tensor_mul(o3, fin3[:, :, 0:D], inv[:].broadcast_to([128, QS, D]))

    # xt load groups: tile 0 alone (so mm1(0) starts after 256KB, not 512KB),
    # then pairs; output store groups: pairs, with the last two tiles single
    # (so the final DMA is small and starts early).
    assert n_tiles >= 4 and n_tiles % 2 == 0
    load_at = {t: 2 for t in range(0, n_tiles, 2)}
    store_group = {}
    for b in range(0, n_tiles - 2, 2):
        store_group[b] = (b, 2)
        store_group[b + 1] = (b, 2)
    store_group[n_tiles - 2] = (n_tiles - 2, 1)
    store_group[n_tiles - 1] = (n_tiles - 1, 1)

    # main loop, software-pipelined one tile deep: mm1/exp of tile t+1 is
    # emitted before mm2/normalize of tile t so the PE keeps feeding ScalarE.
    xt_bufs = {}

    def get_xt(t):
        if t in load_at:
            xt_bufs.clear()
            for i in range(load_at[t]):
                xt_bufs[t + i] = (load_xt(t, load_at[t]), i * TILE_PTS)
        return xt_bufs[t]

    o_bufs = {}

    def put_out(t, e_sb):
        base, width = store_group[t]
        if t == base:
            o_bufs[base] = outp.tile(
                [128, width, QS * D], F32, tag="o2", name=f"o2_{t}"
            )
        o2_t = o_bufs[base]
        mm2_norm(t, e_sb, o2_t, t - base)
        if t == base + width - 1:
            nc.gpsimd.dma_start(
                out_r[base : base + width].rearrange("a p n -> p a n"), o2_t
            )

    xt2, toff = get_xt(0)
    e_prev = mm1_exp(0, xt2, toff)
    for t in range(1, n_tiles + 1):
        if t < n_tiles:
            xt2, toff = get_xt(t)
            e_cur = mm1_exp(t, xt2, toff)
        put_out(t - 1, e_prev)
        if t < n_tiles:
            e_prev = e_cur


def build_nc(n_pts=N_PTS, debug=False):
    nc = bacc.Bacc("TRN2", target_bir_lowering=False, debug=debug, num_devices=N_CORES)
    xt_in = nc.dram_tensor("xt", [D, n_pts], F32R, kind="ExternalInput").ap()
    ct2_in = nc.dram_tensor("ct2", [128, K], F32R, kind="ExternalInput").ap()
    cw_in = nc.dram_tensor("cw", [128, KC * (D + 1)], BF16, kind="ExternalInput").ap()
    out = nc.dram_tensor("out", [n_pts, D], F32, kind="ExternalOutput").ap()
    with tile.TileContext(nc) as tc:
        _kmeans_body(tc, out, xt_in, ct2_in, cw_in, n_pts)
    nc.compile()
    return nc


def _host_xt(x_shard: np.ndarray) -> np.ndarray:
    """[n, 64] -> column-permuted transpose [64, n]:
    xT[d, t*512 + q*128 + j] = x[t*512 + 4j + q, d]."""
    n = x_shard.shape[0]
    xs = x_shard.reshape(n // TILE_PTS, 128, QS, D)
    return np.ascontiguousarray(xs.transpose(3, 0, 2, 1).reshape(D, n))


def _host_consts(centroids: np.ndarray):
    c = centroids.astype(np.float64)
    ct2 = np.concatenate([centroids.T, centroids.T], axis=0).astype(np.float32)
    w = np.exp(-(c * c).sum(-1) / TEMP)  # [K]
    aug = np.concatenate([c * w[:, None], w[:, None]], axis=1)  # [K, 65]
    cw = (
        aug.reshape(KC, 128, D + 1)
        .transpose(1, 0, 2)
        .reshape(128, KC * (D + 1))
        .astype(ml_dtypes.bfloat16)
    )
    return np.ascontiguousarray(ct2), np.ascontiguousarray(cw)


_NC_CACHE = None


def kernel(x: np.ndarray, centroids: np.ndarray) -> np.ndarray:
    global _NC_CACHE
    orig_shape = x.shape
    xf = x.reshape(-1, D).astype(np.float32, copy=False)
    cf = centroids.astype(np.float32, copy=False)
    n_total = xf.shape[0]
    assert n_total == N_CORES * N_PTS, n_total

    if _NC_CACHE is None:
        _NC_CACHE = build_nc()
    nc = _NC_CACHE

    ct2, cw = _host_consts(cf)
    in_maps = [
        {"xt": _host_xt(xf[i * N_PTS : (i + 1) * N_PTS]), "ct2": ct2, "cw": cw}
        for i in range(N_CORES)
    ]
    res = run_bass_kernel_spmd(nc, in_maps, core_ids=list(range(N_CORES)))
    out = np.concatenate([res.results[i]["out"] for i in range(N_CORES)], axis=0)
    return out.reshape(orig_shape).astype(x.dtype, copy=False)
